# revision 44
# baseline (speedup 1.0000x reference)
"""AnalyticGaussianVelocity Trainium2 kernel, 8 NeuronCores.

Math (reference):
    a=t, b=1-t
    logit_n = -(1/(2b^2)) * (|x|^2 - 2a x.y_n + a^2 |y_n|^2)
    v = -(1/b) x + (1 + a/b) * softmax(logit) @ dataset

Device-side per core (dataset sharded along N, padded 50000->50176, 6272/core,
free-dim chunks 11x512+384+256):
    G_n   = x.y_n - (a/2)(|y_n|^2 - 512)          (f32r matmuls + split-bf16 rank-1)
    logit'_n = c1 * G_n, c1 = a/b^2                (per-row scale; per-row constants
                                                    drop out of softmax)
    m     = max_n logit'_n                         (chunked, free-dim reduce)
    P_n   = exp(logit'_n - m), l = sum_n P_n
    S     = P @ ds_shard                           (PE-transposed P tiles, bf16)
Cross-core combine (flash-attention style), on device when COMBINE=True:
    M = AllReduce-max(m);  w = exp(m - M)
    [Sg | lg] = ReduceScatter-add of [w*q2*S | w*l]   (each core gets its B/8 rows)
    v_rows = q1*x_rows + Sg / lg                      (q1=-1/b, q2=1+a/b)
    host concatenates the 8 row-shards.

Padding rows are the constant 2048.0 -> giant |y|^2 -> logit ~ -1e7 -> weight 0.
"""

import numpy as np
import ml_dtypes

import concourse.bass as bass
from concourse import bacc
import concourse.mybir as mybir
import concourse.tile as tile
from concourse.bass_utils import run_bass_kernel_spmd

F32 = mybir.dt.float32
F32R = mybir.dt.float32r
BF16 = mybir.dt.bfloat16
BF = ml_dtypes.bfloat16

B, D, N = 1024, 512, 50000
NCORES = 8
NPAD = 50176                      # 8 * 6272, multiple of 128
NSH = NPAD // NCORES              # 6272 per core
KD = D // 128                     # 4 contraction tiles for logits matmul
BT = B // 128                     # 8 batch tiles
CHUNKS = [512] * 11 + [384, 256]  # free-dim chunks of NSH (all >=256: full-rate f32r)
NK2 = NSH // 128                  # 49 contraction tiles for P @ ds
PADVAL = 2048.0
X = mybir.AxisListType.X


def _build(combine=True):
    nc = bacc.Bacc("TRN2", target_bir_lowering=False, debug=False,
                   num_devices=NCORES, dynamic_dma_scratch_size=512)

    xT = nc.declare_dram_parameter("xT", [KD, 128, B], F32R, isOutput=False)
    dsT = nc.declare_dram_parameter("dsT", [KD, 128, NSH], F32R, isOutput=False)
    dsn = nc.declare_dram_parameter("ds_nat", [NK2, 128, D], BF16, isOutput=False)
    r1l = nc.declare_dram_parameter("r1_lhsT", [3, B], BF16, isOutput=False)
    r1r = nc.declare_dram_parameter("r1_rhs", [3, NSH], BF16, isOutput=False)
    c1d = nc.declare_dram_parameter("c1", [128, BT], F32, isOutput=False)
    idd = nc.declare_dram_parameter("ident", [128, 128], BF16, isOutput=False)
    if combine:
        q2d = nc.declare_dram_parameter("q2", [128, BT], F32, isOutput=False)
        xq1 = nc.declare_dram_parameter("xq1", [128, D], F32, isOutput=False)
        vout = nc.declare_dram_parameter("out", [128, D], F32, isOutput=True)
        S_loc = nc.dram_tensor("S_loc", [B, D], F32)
        mb = nc.dram_tensor("m_bounce", [128, BT], F32)
        mM = nc.dram_tensor("m_red", [128, BT], F32, addr_space="Shared")
        rs_in = nc.dram_tensor("rs_in", [B, D + 1], F32)
        rs_out = nc.dram_tensor("rs_out", [128, D + 1], F32)
    else:
        S_out = nc.declare_dram_parameter("S_out", [B, D], F32, isOutput=True)
        m_out = nc.declare_dram_parameter("m_out", [B, 1], F32, isOutput=True)
        l_out = nc.declare_dram_parameter("l_out", [B, 1], F32, isOutput=True)

    nch = len(CHUNKS)
    coff = np.concatenate([[0], np.cumsum(CHUNKS)])

    with tile.TileContext(nc) as tc:
        with (
            tc.tile_pool(name="res", bufs=1) as res,
            tc.tile_pool(name="gpool", bufs=18) as gpool,
            tc.tile_pool(name="ppool", bufs=3) as ppool,
            tc.tile_pool(name="small", bufs=2) as small,
            tc.tile_pool(name="ptsb", bufs=6) as ptsb_pool,
            tc.tile_pool(name="sbout", bufs=2) as sbout,
            tc.tile_pool(name="fin", bufs=3) as fin,
            tc.tile_pool(name="gps", bufs=3, space="PSUM") as gps,
            tc.tile_pool(name="spsum", bufs=2, space="PSUM") as spsum,
            tc.tile_pool(name="tpsum", bufs=3, space="PSUM") as tpsum,
        ):
            # ---- residents (DMA in first-use order; smalls off the sync queue) ----
            r1l_t = res.tile([3, B], BF16, tag="r1l")
            nc.gpsimd.dma_start(r1l_t[:], r1l[:])
            r1r_t = res.tile([3, NSH], BF16, tag="r1r")
            nc.gpsimd.dma_start(r1r_t[:], r1r[:])
            c1_t = res.tile([128, BT], F32, tag="c1")
            nc.gpsimd.dma_start(c1_t[:], c1d[:])
            id_t = res.tile([128, 128], BF16, tag="ident")
            nc.gpsimd.dma_start(id_t[:], idd[:])

            xT_r = res.tile([128, KD, B], F32R, tag="xT_r")
            xT_re = xT.ap().rearrange("k p b -> p k b")
            nc.sync.dma_start(xT_r[:, :, 0:128], xT_re[:, :, 0:128])
            dsT_r = res.tile([128, KD, NSH], F32R, tag="dsT_r")
            for c, w in enumerate(CHUNKS):
                o = int(coff[c])
                for k in range(KD):
                    nc.sync.dma_start(dsT_r[:, k, o:o + w], dsT.ap()[k, :, o:o + w])
                if c == 0 and BT > 1:
                    nc.sync.dma_start(xT_r[:, :, 128:256], xT_re[:, :, 128:256])
            for i in range(2, BT):
                nc.sync.dma_start(xT_r[:, :, i * 128:(i + 1) * 128],
                                  xT_re[:, :, i * 128:(i + 1) * 128])

            dnat_t = res.tile([128, NK2, D], BF16, tag="dnat")
            kt_groups = np.array_split(np.arange(NK2), min(8, NK2))
            for grp in kt_groups:
                k0, k1 = int(grp[0]), int(grp[-1]) + 1
                nc.sync.dma_start(
                    dnat_t[:, k0:k1, :],
                    dsn.ap()[k0:k1].rearrange("k p d -> p k d"),
                )

            m_sb = res.tile([128, BT], F32, tag="m_sb")
            l_sb = res.tile([128, BT], F32, tag="l_sb")
            if combine:
                q2_t = res.tile([128, BT], F32, tag="q2")
                nc.sync.dma_start(q2_t[:], q2d[:])
                M_sb = res.tile([128, BT], F32, tag="M_sb")
                wN = res.tile([128, BT], F32, tag="wN")
                wD = res.tile([128, BT], F32, tag="wD")

            state = {}

            def emit_mm1_chunk(i, c, w):
                o = int(coff[c])
                gch, gmax = state[i][:2]
                g_ps = gps.tile([128, 512], F32, tag="gps")
                for k in range(KD):
                    nc.tensor.matmul(
                        g_ps[:, :w],
                        xT_r[:, k, i * 128:(i + 1) * 128],
                        dsT_r[:, k, o:o + w],
                        start=(k == 0), stop=False,
                    )
                nc.tensor.matmul(
                    g_ps[:, :w],
                    r1l_t[:, i * 128:(i + 1) * 128],
                    r1r_t[:, o:o + w],
                    start=False, stop=True,
                )
                G_c = gpool.tile([128, 512], F32, tag="G")
                nc.scalar.activation(G_c[:, :w], g_ps[:, :w],
                                     mybir.ActivationFunctionType.Copy)
                gch[c] = G_c

            def emit_max(i, c):
                gch, gmax = state[i][:2]
                w = CHUNKS[c]
                nc.vector.reduce_max(gmax[:, c:c + 1], gch[c][:, :w], axis=X,
                                     op=mybir.AluOpType.max)

            def emit_exp_head(i):
                gch, gmax = state[i]
                gm = small.tile([128, 1], F32, tag="gm")
                nc.vector.reduce_max(gm[:], gmax[:], axis=X, op=mybir.AluOpType.max)
                nc.vector.tensor_mul(m_sb[:, i:i + 1], gm[:], c1_t[:, i:i + 1])
                nb = small.tile([128, 1], F32, tag="nb")
                nc.vector.tensor_scalar_mul(nb[:], m_sb[:, i:i + 1], -1.0)
                if not combine:
                    nc.sync.dma_start(m_out[i * 128:(i + 1) * 128, :],
                                      m_sb[:, i:i + 1])
                lparts = small.tile([128, nch], F32, tag="lp")
                S_ps = spsum.tile([128, D], F32, tag="S")
                state[i] = (gch, gmax, nb, lparts, S_ps)

            pexp = {}

            def emit_exp_chunk(i, c):
                gch, gmax, nb, lparts, S_ps = state[i]
                w = CHUNKS[c]
                G_c = gch.pop(c)
                P_c = ppool.tile([128, 512], BF16, tag="P")
                nc.scalar.activation(
                    P_c[:, :w], G_c[:, :w],
                    mybir.ActivationFunctionType.Exp,
                    bias=nb[:], scale=c1_t[:, i:i + 1],
                    accum_out=lparts[:, c:c + 1],
                )
                pexp[(i, c)] = P_c

            def emit_mm2_chunk(i, c, w):
                gch, gmax, nb, lparts, S_ps = state[i]
                o = int(coff[c])
                P_c = pexp.pop((i, c))
                pts = []
                for j0 in range(0, w, 128):
                    pt_ps = tpsum.tile([128, 128], BF16, tag="pt")
                    nc.tensor.transpose(pt_ps[:], P_c[:, j0:j0 + 128], id_t[:])
                    pt_sb = ptsb_pool.tile([128, 128], BF16, tag="ptsb")
                    nc.vector.tensor_copy(pt_sb[:], pt_ps[:])
                    pts.append(pt_sb)
                for n, j0 in enumerate(range(0, w, 128)):
                    kt = (o + j0) // 128
                    nc.tensor.matmul(S_ps[:], pts[n][:], dnat_t[:, kt, :],
                                     start=(kt == 0), stop=(kt == NK2 - 1))

            def emit_mm2_tail(i):
                _, _, _, lparts, S_ps = state.pop(i)
                nc.vector.reduce_sum(l_sb[:, i:i + 1], lparts[:], axis=X,
                                     op=mybir.AluOpType.add)
                if not combine:
                    nc.sync.dma_start(l_out[i * 128:(i + 1) * 128, :],
                                      l_sb[:, i:i + 1])
                S_sb = sbout.tile([128, D], F32, tag="S_sb")
                nc.vector.tensor_copy(S_sb[:], S_ps[:])
                dst = S_loc if combine else S_out
                nc.sync.dma_start(dst[i * 128:(i + 1) * 128, :], S_sb[:])

            def emit_m_collective():
                nc.sync.dma_start(mb[:], m_sb[:])
                nc.gpsimd.collective_compute(
                    "AllReduce", mybir.AluOpType.max,
                    replica_groups=[list(range(NCORES))],
                    ins=[mb.ap()], outs=[mM.ap()],
                )
                nc.sync.dma_start(M_sb[:], mM[:])
                dcol = small.tile([128, BT], F32, tag="dcol")
                nc.vector.tensor_sub(dcol[:], m_sb[:], M_sb[:])
                nc.scalar.activation(wD[:], dcol[:],
                                     mybir.ActivationFunctionType.Exp)
                nc.vector.tensor_mul(wN[:], wD[:], q2_t[:])

            def emit_rescale(i):
                F = fin.tile([128, D + 1], F32, tag="fin")
                nc.sync.dma_start(F[:, :D], S_loc[i * 128:(i + 1) * 128, :])
                nc.vector.tensor_scalar_mul(F[:, :D], F[:, :D], wN[:, i:i + 1])
                nc.vector.tensor_mul(F[:, D:D + 1], l_sb[:, i:i + 1],
                                     wD[:, i:i + 1])
                nc.sync.dma_start(rs_in[i * 128:(i + 1) * 128, :], F[:])

            def emit_final():
                nc.gpsimd.collective_compute(
                    "ReduceScatter", mybir.AluOpType.add,
                    replica_groups=[list(range(NCORES))],
                    ins=[rs_in.ap()], outs=[rs_out.ap()],
                )
                R = fin.tile([128, D + 1], F32, tag="fin")
                nc.sync.dma_start(R[:], rs_out[:])
                rec = small.tile([128, 1], F32, tag="rec")
                nc.vector.reciprocal(rec[:], R[:, D:D + 1])
                nc.vector.tensor_scalar_mul(R[:, :D], R[:, :D], rec[:])
                Xf = fin.tile([128, D + 1], F32, tag="fin")
                nc.sync.dma_start(Xf[:, :D], xq1[:])
                V = fin.tile([128, D + 1], F32, tag="fin")
                nc.vector.tensor_add(V[:, :D], R[:, :D], Xf[:, :D])
                nc.sync.dma_start(vout[:], V[:, :D])

            def alloc_tile_state(i):
                gmax = small.tile([128, nch], F32, tag="gmax")
                state[i] = ({}, gmax)

            # software-pipelined: mm1 of tile i interleaved chunk-by-chunk
            # with exp/transpose/mm2 of tile i-1; exp leads its PE consumer by
            # one chunk, reduce_max lags its producer by one chunk
            PRE = 5 if BT > 1 else 0   # tile-1 chunks pulled into startup
            alloc_tile_state(0)
            if BT > 1:
                alloc_tile_state(1)
            for c, w in enumerate(CHUNKS):
                emit_mm1_chunk(0, c, w)
                if c < PRE:
                    emit_mm1_chunk(1, c, w)
                if c > 0:
                    emit_max(0, c - 1)
            emit_max(0, nch - 1)
            for i in range(1, BT):
                if i not in state:
                    alloc_tile_state(i)
                emit_exp_head(i - 1)
                emit_exp_chunk(i - 1, 0)
                sh = PRE if i == 1 else 0   # step-1 mm1 chunks shifted by PRE
                for k in range(sh):
                    emit_max(i, k)
                for c, w in enumerate(CHUNKS):
                    if c + 1 < nch:
                        emit_exp_chunk(i - 1, c + 1)
                    emit_mm2_chunk(i - 1, c, w)
                    if c + sh < nch:
                        emit_mm1_chunk(i, c + sh, CHUNKS[c + sh])
                    if c > 0 and c - 1 + sh < nch:
                        emit_max(i, c - 1 + sh)
                emit_max(i, nch - 1)
                emit_mm2_tail(i - 1)
            emit_exp_head(BT - 1)
            if combine:
                emit_m_collective()
            emit_exp_chunk(BT - 1, 0)
            for c, w in enumerate(CHUNKS):
                if c + 1 < nch:
                    emit_exp_chunk(BT - 1, c + 1)
                emit_mm2_chunk(BT - 1, c, w)
                if combine and c < BT - 1:
                    emit_rescale(c)
            emit_mm2_tail(BT - 1)
            if combine:
                emit_rescale(BT - 1)
                emit_final()

    nc.compile()
    return nc


_NC_CACHE = {}


def _get_nc(combine=True):
    if combine not in _NC_CACHE:
        _NC_CACHE[combine] = _build(combine)
    return _NC_CACHE[combine]


def _split_bf16(v):
    hi = v.astype(np.float32).astype(BF)
    lo = (v.astype(np.float64) - hi.astype(np.float64)).astype(np.float32).astype(BF)
    return hi, lo


def _prep_inputs(x_t, t, dataset, combine=True):
    x_t = np.asarray(x_t, dtype=np.float32)
    t = np.asarray(t, dtype=np.float32)
    dataset = np.asarray(dataset, dtype=np.float32)

    a = t.astype(np.float64)
    b = 1.0 - a
    c1 = np.ascontiguousarray(
        (a / (b * b)).astype(np.float32).reshape(BT, 128).T)
    u = -a / 2.0
    u_hi, u_lo = _split_bf16(u)
    r1_lhsT = np.stack([u_hi, u_lo, u_hi]).astype(BF)          # (3, B)

    dsp = np.full((NPAD, D), PADVAL, dtype=np.float32)
    dsp[:N] = dataset
    dsnc = (dsp.astype(np.float64) ** 2).sum(1) - float(D)      # centered |y|^2
    v_hi, v_lo = _split_bf16(dsnc)
    r1_rhs_full = np.stack([v_hi, v_hi, v_lo]).astype(BF)       # (3, NPAD)

    xT = np.ascontiguousarray(x_t.T).reshape(KD, 128, B)
    dsT_full = np.ascontiguousarray(dsp.T)                      # (D, NPAD)
    ds_bf = dsp.astype(BF)                                      # (NPAD, D)

    ident = np.eye(128, dtype=np.float32).astype(BF)
    q2 = np.ascontiguousarray(
        (1.0 + a / b).astype(np.float32).reshape(BT, 128).T)
    x_q1 = ((-1.0 / b)[:, None] * x_t.astype(np.float64)).astype(np.float32)

    in_maps = []
    for c in range(NCORES):
        sl = slice(c * NSH, (c + 1) * NSH)
        im = {
            "xT": xT,
            "dsT": np.ascontiguousarray(dsT_full[:, sl]).reshape(KD, 128, NSH),
            "ds_nat": np.ascontiguousarray(ds_bf[sl]).reshape(NK2, 128, D),
            "r1_lhsT": r1_lhsT,
            "r1_rhs": np.ascontiguousarray(r1_rhs_full[:, sl]),
            "c1": c1,
            "ident": ident,
        }
        if combine:
            im["q2"] = q2
            im["xq1"] = x_q1[c * 128:(c + 1) * 128, :]
        in_maps.append(im)
    return in_maps


def _combine_host(results, x_t, t):
    a = t.astype(np.float64)
    b = 1.0 - a
    m_c = np.stack([np.asarray(r["m_out"], dtype=np.float64)[:, 0]
                    for r in results])                          # (8, B)
    l_c = np.stack([np.asarray(r["l_out"], dtype=np.float64)[:, 0]
                    for r in results])                          # (8, B)
    S_c = np.stack([np.asarray(r["S_out"], dtype=np.float64)
                    for r in results])                          # (8, B, D)
    M = m_c.max(0)
    w = np.exp(m_c - M)                                         # (8, B)
    S = np.einsum("cb,cbd->bd", w, S_c)
    L = (w * l_c).sum(0)
    wd = S / L[:, None]
    v = (-1.0 / b)[:, None] * x_t.astype(np.float64) \
        + (1.0 + a / b)[:, None] * wd
    return v.astype(np.float32)


def run_full(x_t, t, dataset, trace=False, combine=False):
    nc = _get_nc(combine)
    in_maps = _prep_inputs(x_t, t, dataset, combine=combine)
    res = run_bass_kernel_spmd(nc, in_maps, core_ids=list(range(NCORES)),
                               trace=trace)
    if combine:
        v = np.concatenate([np.asarray(r["out"]) for r in res.results], axis=0)
    else:
        v = _combine_host(res.results, np.asarray(x_t, np.float32),
                          np.asarray(t, np.float32))
    return v, res


def kernel(x_t, t, dataset):
    v, _ = run_full(x_t, t, dataset)
    return v
